# revision 7
# baseline (speedup 1.0000x reference)
"""Trainium2 Bass kernel for nn_ARIG_Fusion (dual sigmoid gating + proj + BatchNorm + LIF).

Strategy (8 NeuronCores, SPMD):
  - Shard batch B=32 into 8 shards of 4. Each core handles rows (t, b_loc, n)
    = 4*4*1024 = 16384 rows of C=256 channels.
  - All tensors live on-chip in TRANSPOSED layout [channel, row]: the host
    pre-transposes inputs and post-transposes outputs, so the device never
    transposes anything and all DMAs are wide contiguous runs.
  - Gate GEMMs run in float32r (11-bit mantissa, 1 cyc/row on the PE); the
    sigmoid + downstream projection attenuate the rounding. The projection
    GEMM runs in full fp32 (4 cyc/row) since BatchNorm output feeds a
    spike threshold that is sensitive to error.
  - BatchNorm stats: per-tile bn_stats -> bn_aggr locally, then a tiny
    [128,4] AllReduce across the 8 cores combines (mean, E[x^2]).
  - LIF over T=4 uses a rescaled state U_t = v_t/(1-tau)^t so each step is
    one scalar_tensor_tensor + one tensor_scalar compare (no 0.5*v op).
"""

import math

import numpy as np

T, B, N, C = 4, 32, 1024, 256
NCORES = 8
BL = B // NCORES          # 4 batches per core
R = T * BL * N            # 16384 rows per core
RT = BL * N               # 4096 rows per t-slice
F = 512                   # gating tile columns
NT = R // F               # 32 gating tiles
CC = 1024                 # LIF column chunk
NQ = RT // CC             # 4 LIF chunks per t-slice
EPS = 1e-5
V_TH = 1.0

_program_cache = {}

# tuning knobs (read at trace time)
GIN_BUFS = 2
GW_BUFS = 2
GATE_BUFS = 2
PG_BUFS = 1
PO_BUFS = 2
LIF_BUFS = 2
LIF_POOL_Q = ()          # q indices whose U/W ops go to gpsimd
S_ON_DVE = False
PHASES = ("gate", "fin", "lif")


def _build_program(tau_inv: float, reps: int = 1, single_core: bool = False):
    do_fin = "fin" in PHASES
    do_lif = "lif" in PHASES
    import concourse.bacc as bacc
    import concourse.tile as tile
    from concourse import mybir

    f32 = mybir.dt.float32
    f32r = mybir.dt.float32r
    Alu = mybir.AluOpType
    Act = mybir.ActivationFunctionType

    # LIF rescaling: U_t = v_t / (1-tau)^t;  U_t = W_{t-1} + alpha_t * y_t
    # with W = U * (1 - spike) carried negated (Wn = (s-1)*U) so both steps
    # fit scalar_tensor_tensor.
    one_m = 1.0 - tau_inv
    alphas = [tau_inv / (one_m ** t) for t in range(T)]
    ths = [V_TH / (one_m ** t) for t in range(T)]

    nc = bacc.Bacc("TRN2", target_bir_lowering=False, debug=False,
                   num_devices=1 if single_core else NCORES)

    at_d = nc.dram_tensor("at", [2, 128, R], f32, kind="ExternalInput")
    lt_d = nc.dram_tensor("lt", [2, 128, R], f32, kind="ExternalInput")
    w1_d = nc.dram_tensor("w1", [128, 2, 2, 128], f32r, kind="ExternalInput")
    w2_d = nc.dram_tensor("w2", [128, 2, 2, 128], f32r, kind="ExternalInput")
    w3_d = nc.dram_tensor("w3", [128, 2, 2, 128], f32, kind="ExternalInput")
    pp_d = nc.dram_tensor("pp", [128, 10], f32, kind="ExternalInput")
    sp_d = nc.dram_tensor("sp", [2, 128, R], f32, kind="ExternalOutput")

    with tile.TileContext(nc) as tc:
      for _rep in range(reps):
        with tc.tile_pool(name="singles", bufs=1) as singles:
            w1s = singles.tile([128, 2, 2, 128], f32r)
            w2s = singles.tile([128, 2, 2, 128], f32r)
            w3s = singles.tile([128, 2, 2, 128], f32)
            pps = singles.tile([128, 10], f32)
            outb = singles.tile([128, 2, R], f32)
            stb = singles.tile([128, 2, NT, 6], f32)
            nc.sync.dma_start(w1s[:], w1_d[:, :, :, :])
            nc.sync.dma_start(w2s[:], w2_d[:, :, :, :])
            nc.sync.dma_start(w3s[:], w3_d[:, :, :, :])
            nc.sync.dma_start(pps[:], pp_d[:, :])

            # ---------------- gating + projection + stats ----------------
            with (
                tc.tile_pool(name="gin", bufs=GIN_BUFS) as gin,
                tc.tile_pool(name="gw", bufs=GW_BUFS) as gw,
                tc.tile_pool(name="gate", bufs=GATE_BUFS) as gatep,
                tc.tile_pool(name="pg", bufs=PG_BUFS, space="PSUM") as pg,
                tc.tile_pool(name="po", bufs=PO_BUFS, space="PSUM") as po,
            ):
                for i in range(NT):
                    sl = slice(i * F, (i + 1) * F)
                    a = [gin.tile([128, F], f32, tag=f"a{j}", name=f"a{j}") for j in (0, 1)]
                    l = [gin.tile([128, F], f32, tag=f"l{j}", name=f"l{j}") for j in (0, 1)]
                    for j in (0, 1):
                        nc.sync.dma_start(a[j][:], at_d[j, :, sl])
                        nc.sync.dma_start(l[j][:], lt_d[j, :, sl])
                    ar = [gw.tile([128, F], f32r, tag=f"ar{j}", name=f"ar{j}") for j in (0, 1)]
                    lr = [gw.tile([128, F], f32r, tag=f"lr{j}", name=f"lr{j}") for j in (0, 1)]
                    for j in (0, 1):
                        nc.gpsimd.tensor_copy(ar[j][:], a[j][:])
                        nc.gpsimd.tensor_copy(lr[j][:], l[j][:])
                    gL, gA = [], []
                    for j in (0, 1):
                        p1 = pg.tile([128, F], f32, tag=f"g1{j}", name=f"pg1{j}")
                        nc.tensor.matmul(p1[:], w1s[:, 0, j, :], ar[0][:],
                                         start=True, stop=False)
                        nc.tensor.matmul(p1[:], w1s[:, 1, j, :], ar[1][:],
                                         start=False, stop=True)
                        g = gatep.tile([128, F], f32, tag=f"gL{j}", name=f"gL{j}")
                        nc.scalar.activation(g[:], p1[:], Act.Sigmoid,
                                             bias=pps[:, 0 + j:1 + j])
                        gL.append(g)
                    for j in (0, 1):
                        p2 = pg.tile([128, F], f32, tag=f"g2{j}", name=f"pg2{j}")
                        nc.tensor.matmul(p2[:], w2s[:, 0, j, :], lr[0][:],
                                         start=True, stop=False)
                        nc.tensor.matmul(p2[:], w2s[:, 1, j, :], lr[1][:],
                                         start=False, stop=True)
                        g = gatep.tile([128, F], f32, tag=f"gA{j}", name=f"gA{j}")
                        nc.scalar.activation(g[:], p2[:], Act.Sigmoid,
                                             bias=pps[:, 2 + j:3 + j])
                        gA.append(g)
                    fu = []
                    for j in (0, 1):
                        v1 = gw.tile([128, F], f32, tag=f"v1{j}", name=f"v1{j}")
                        v2 = gw.tile([128, F], f32, tag=f"v2{j}", name=f"v2{j}")
                        nc.vector.tensor_mul(v1[:], a[j][:], gA[j][:])
                        nc.gpsimd.tensor_mul(v2[:], l[j][:], gL[j][:])
                        nc.vector.tensor_add(v1[:], v1[:], v2[:])
                        fu.append(v1)
                    for j in (0, 1):
                        p3 = po.tile([128, F], f32, tag=f"o{j}", name=f"po{j}")
                        nc.tensor.matmul(p3[:], w3s[:, 0, j, :], fu[0][:],
                                         start=True, stop=False)
                        nc.tensor.matmul(p3[:], w3s[:, 1, j, :], fu[1][:],
                                         start=False, stop=True)
                        nc.scalar.activation(outb[:, j, sl], p3[:],
                                             Act.Identity,
                                             bias=pps[:, 4 + j:5 + j])
                        nc.vector.bn_stats(stb[:, j, i, :], outb[:, j, sl])

            # ---------------- stats finalize + all-reduce ----------------
            if not do_fin:
                continue
            with (
                tc.tile_pool(name="fin", bufs=1) as fin,
                tc.tile_pool(name="dramp", bufs=1, space="DRAM") as dramp,
            ):
                mv = fin.tile([128, 2, 2], f32)
                ccs = fin.tile([128, 4], f32)
                for j in (0, 1):
                    nc.vector.bn_aggr(mv[:, j, :], stb[:, j, :, :])
                    nc.vector.tensor_copy(ccs[:, 2 * j:2 * j + 1], mv[:, j, 0:1])
                    nc.vector.tensor_mul(ccs[:, 2 * j + 1:2 * j + 2],
                                         mv[:, j, 0:1], mv[:, j, 0:1])
                    nc.vector.tensor_add(ccs[:, 2 * j + 1:2 * j + 2],
                                         ccs[:, 2 * j + 1:2 * j + 2],
                                         mv[:, j, 1:2])
                if single_core:
                    cg = fin.tile([128, 4], f32)
                    nc.vector.tensor_scalar(cg[:], ccs[:], float(NCORES),
                                            None, Alu.mult)
                else:
                    cc_in = dramp.tile([128, 4], f32)
                    cc_out = dramp.tile([128, 4], f32)
                    nc.gpsimd.dma_start(cc_in[:], ccs[:])
                    nc.gpsimd.collective_compute(
                        "AllReduce", Alu.add,
                        replica_groups=[list(range(NCORES))],
                        ins=[cc_in.opt()], outs=[cc_out.opt()],
                    )
                    cg = fin.tile([128, 4], f32)
                    nc.gpsimd.dma_start(cg[:], cc_out[:])

                mean = fin.tile([128, 2], f32)
                varp = fin.tile([128, 2], f32)
                sc = fin.tile([128, 2], f32)
                sh = fin.tile([128, 2], f32)
                t1 = fin.tile([128, 2], f32)
                t2 = fin.tile([128, 2], f32)
                r0 = fin.tile([128, 2], f32)
                for j in (0, 1):
                    jm = slice(j, j + 1)
                    nc.vector.tensor_scalar(mean[:, jm], cg[:, 2 * j:2 * j + 1],
                                            1.0 / NCORES, None, Alu.mult)
                    # varp = E[x^2] - mean^2 + eps
                    nc.vector.tensor_scalar(varp[:, jm],
                                            cg[:, 2 * j + 1:2 * j + 2],
                                            1.0 / NCORES, None, Alu.mult)
                    nc.vector.tensor_mul(t1[:, jm], mean[:, jm], mean[:, jm])
                    nc.vector.tensor_sub(varp[:, jm], varp[:, jm], t1[:, jm])
                    nc.vector.tensor_scalar(varp[:, jm], varp[:, jm], EPS,
                                            None, Alu.add)
                # r0 = 1/sqrt(varp), via ACT sqrt + reciprocal + 2 Newton steps
                nc.scalar.activation(r0[:], varp[:], Act.Sqrt)
                nc.vector.reciprocal(r0[:], r0[:])
                for _ in range(2):
                    nc.vector.tensor_mul(t1[:], r0[:], r0[:])
                    nc.vector.tensor_mul(t2[:], t1[:], varp[:])
                    nc.vector.tensor_scalar(t2[:], t2[:], -0.5, 1.5,
                                            Alu.mult, Alu.add)
                    nc.vector.tensor_mul(r0[:], r0[:], t2[:])
                for j in (0, 1):
                    jm = slice(j, j + 1)
                    nc.vector.tensor_mul(sc[:, jm], pps[:, 6 + j:7 + j],
                                         r0[:, jm])
                    nc.vector.tensor_mul(t1[:, jm], mean[:, jm], sc[:, jm])
                    nc.vector.tensor_sub(sh[:, jm], pps[:, 8 + j:9 + j],
                                         t1[:, jm])

                # per-t pre-scaled BN affine: ay_t = alpha_t*(sc*out+sh)
                asc = fin.tile([128, T, 2], f32)
                ash = fin.tile([128, T, 2], f32)
                for t in range(T):
                    for j in (0, 1):
                        nc.vector.tensor_scalar(asc[:, t, j:j + 1], sc[:, j:j + 1],
                                                alphas[t], None, Alu.mult)
                        nc.vector.tensor_scalar(ash[:, t, j:j + 1], sh[:, j:j + 1],
                                                alphas[t], None, Alu.mult)

                # ---------------- LIF scan + spike output ----------------
                if not do_lif:
                    continue
                with tc.tile_pool(name="lif", bufs=LIF_BUFS) as lifp:
                    for q in range(NQ):
                        for j in (0, 1):
                            veng = nc.gpsimd if q in LIF_POOL_Q else nc.vector
                            seng = nc.vector if S_ON_DVE else nc.gpsimd
                            wprev = None
                            for t in range(T):
                                ysl = outb[:, j, t * RT + q * CC:
                                           t * RT + (q + 1) * CC]
                                ay = lifp.tile([128, CC], f32, tag=f"ay{j}", name=f"ay{j}")
                                nc.scalar.activation(
                                    ay[:], ysl, Act.Identity,
                                    bias=ash[:, t, j:j + 1],
                                    scale=asc[:, t, j:j + 1])
                                if t == 0:
                                    u = ay
                                else:
                                    # U = ay - Wn  (Wn = -W)
                                    u = lifp.tile([128, CC], f32, tag=f"U{j}", name=f"U{j}")
                                    veng.tensor_sub(u[:], ay[:], wprev[:])
                                s = lifp.tile([128, CC], f32, tag=f"s{j}", name=f"s{j}")
                                seng.tensor_scalar(
                                    s[:], u[:], ths[t], None, Alu.is_ge)
                                nc.sync.dma_start(
                                    sp_d[j, :, t * RT + q * CC:
                                         t * RT + (q + 1) * CC], s[:])
                                if t < T - 1:
                                    wn = lifp.tile([128, CC], f32, tag=f"W{j}", name=f"Wn{j}")
                                    veng.scalar_tensor_tensor(
                                        wn[:], s[:], 1.0, u[:],
                                        Alu.subtract, Alu.mult)
                                    wprev = wn

    nc.compile()
    return nc


def _get_program(tau_inv: float, reps: int = 1, single_core: bool = False):
    key = (round(float(tau_inv), 12), reps, single_core)
    if key not in _program_cache:
        _program_cache[key] = _build_program(float(tau_inv), reps, single_core)
    return _program_cache[key]


def _shard_transpose(x):
    # [T,B,N,C] -> [cores, 2, 128, R] with rows ordered (t, b_loc, n)
    v = x.reshape(T, NCORES, BL, N, C)
    v = np.transpose(v, (1, 4, 0, 2, 3))
    return np.ascontiguousarray(v).reshape(NCORES, 2, 128, R)


def _prep_w(w):
    # lhsT chunks [p, k, j, q]: W.T viewed as [k,128p][j,128q]
    wt = np.ascontiguousarray(w.T).reshape(2, 128, 2, 128)
    return np.ascontiguousarray(wt.transpose(1, 0, 2, 3))


def _two(vec):
    return np.ascontiguousarray(vec.reshape(2, 128).T)


def kernel(**inputs):
    from concourse.bass_utils import run_bass_kernel_spmd

    trace = bool(inputs.pop("_trace", False))

    x_attn = np.asarray(inputs["x_attn"], dtype=np.float32)
    x_lsm = np.asarray(inputs["x_lsm"], dtype=np.float32)
    W_att = np.asarray(inputs["W_att"], dtype=np.float32)
    b_att = np.asarray(inputs["b_att"], dtype=np.float32)
    W_lsm = np.asarray(inputs["W_lsm"], dtype=np.float32)
    b_lsm = np.asarray(inputs["b_lsm"], dtype=np.float32)
    W_proj = np.asarray(inputs["W_proj"], dtype=np.float32)
    b_proj = np.asarray(inputs["b_proj"], dtype=np.float32)
    gamma = np.asarray(inputs["gamma"], dtype=np.float32)
    beta = np.asarray(inputs["beta"], dtype=np.float32)
    lif_w = float(np.asarray(inputs["lif_w"], dtype=np.float32))

    tau_inv = float(np.float32(1.0 / (1.0 + math.exp(-lif_w))))
    nc = _get_program(tau_inv)

    at = _shard_transpose(x_attn)
    lt = _shard_transpose(x_lsm)
    w1 = _prep_w(W_att)
    w2 = _prep_w(W_lsm)
    w3 = _prep_w(W_proj)
    pp = np.concatenate(
        [_two(b_att), _two(b_lsm), _two(b_proj), _two(gamma), _two(beta)],
        axis=1)

    in_maps = [
        {"at": at[s], "lt": lt[s], "w1": w1, "w2": w2, "w3": w3, "pp": pp}
        for s in range(NCORES)
    ]
    res = run_bass_kernel_spmd(nc, in_maps, core_ids=list(range(NCORES)),
                               trace=trace)
    kernel.last_results = res

    S = np.stack([r["sp"] for r in res.results]).reshape(
        NCORES, 2, 128, T, BL, N)
    out = np.transpose(S, (3, 0, 4, 5, 1, 2))
    return np.ascontiguousarray(out).reshape(T, B, N, C)


# revision 14
# speedup vs baseline: 2.2523x; 2.2523x over previous
"""Trainium2 Bass kernel for nn_ARIG_Fusion (dual sigmoid gating + proj + BatchNorm + LIF).

Strategy (8 NeuronCores, SPMD):
  - Shard batch B=32 into 8 shards of 4. Each core handles rows (t, b_loc, n)
    = 4*4*1024 = 16384 rows of C=256 channels.
  - All tensors live on-chip in TRANSPOSED layout [channel, row]: the host
    pre-transposes inputs and post-transposes outputs, so the device never
    transposes anything and all DMAs are wide contiguous runs.
  - Gate GEMMs run in float32r (11-bit mantissa, 1 cyc/row on the PE); the
    sigmoid + downstream projection attenuate the rounding. The projection
    GEMM runs in full fp32 (4 cyc/row) since BatchNorm output feeds a
    spike threshold that is sensitive to error.
  - BatchNorm stats: per-tile bn_stats -> bn_aggr locally, then a tiny
    [128,4] AllReduce across the 8 cores combines (mean, E[x^2]).
  - LIF over T=4 uses a rescaled state U_t = v_t/(1-tau)^t so each step is
    one scalar_tensor_tensor + one tensor_scalar compare (no 0.5*v op).
"""

import math

import numpy as np

T, B, N, C = 4, 32, 1024, 256
NCORES = 8
BL = B // NCORES          # 4 batches per core
R = T * BL * N            # 16384 rows per core
RT = BL * N               # 4096 rows per t-slice
F = 512                   # gating tile columns
NT = R // F               # 32 gating tiles
CC = 1024                 # LIF column chunk (knob; NQ derived in-build)
EPS = 1e-5
V_TH = 1.0

_program_cache = {}

# tuning knobs (read at trace time)
GIN_BUFS = 3
GW_BUFS = 2
GATE_BUFS = 2
PG_BUFS = 1
PO_BUFS = 2
LIF_BUFS = 2
LIF_POOL_Q = ()          # q indices whose U/W ops go to gpsimd
S_ON_DVE = False
BN_FROM_PSUM = False
AY_BUFS = 2
AR_ENGINE = "dve"        # gpsimd | act | dve
LR_ENGINE = "dve"
PHASES = ("gate", "fin", "lif")


def _build_program(tau_inv: float, reps: int = 1, single_core: bool = False):
    do_fin = "fin" in PHASES
    do_lif = "lif" in PHASES
    cc = CC
    nq = RT // cc
    import concourse.bacc as bacc
    import concourse.tile as tile
    from concourse import mybir

    f32 = mybir.dt.float32
    f32r = mybir.dt.float32r
    Alu = mybir.AluOpType
    Act = mybir.ActivationFunctionType

    # LIF rescaling: U_t = v_t / (1-tau)^t;  U_t = W_{t-1} + alpha_t * y_t
    # with W = U * (1 - spike) carried negated (Wn = (s-1)*U) so both steps
    # fit scalar_tensor_tensor.
    one_m = 1.0 - tau_inv
    alphas = [tau_inv / (one_m ** t) for t in range(T)]
    ths = [V_TH / (one_m ** t) for t in range(T)]

    nc = bacc.Bacc("TRN2", target_bir_lowering=False, debug=False,
                   num_devices=1 if single_core else NCORES)

    at_d = nc.dram_tensor("at", [2, 128, R], f32, kind="ExternalInput")
    lt_d = nc.dram_tensor("lt", [2, 128, R], f32, kind="ExternalInput")
    w1_d = nc.dram_tensor("w1", [128, 2, 2, 128], f32r, kind="ExternalInput")
    w2_d = nc.dram_tensor("w2", [128, 2, 2, 128], f32r, kind="ExternalInput")
    w3_d = nc.dram_tensor("w3", [128, 2, 2, 128], f32, kind="ExternalInput")
    pp_d = nc.dram_tensor("pp", [128, 10], f32, kind="ExternalInput")
    sp_d = nc.dram_tensor("sp", [2, 128, R], f32, kind="ExternalOutput")

    with tile.TileContext(nc) as tc:
      for _rep in range(reps):
        with tc.tile_pool(name="singles", bufs=1) as singles:
            w1s = singles.tile([128, 2, 2, 128], f32r)
            w2s = singles.tile([128, 2, 2, 128], f32r)
            w3s = singles.tile([128, 2, 2, 128], f32)
            pps = singles.tile([128, 10], f32)
            outb = singles.tile([128, 2, R], f32)
            stb = singles.tile([128, 2, NT, 6], f32)
            nc.sync.dma_start(w1s[:], w1_d[:, :, :, :])
            nc.sync.dma_start(w2s[:], w2_d[:, :, :, :])
            nc.sync.dma_start(w3s[:], w3_d[:, :, :, :])
            nc.sync.dma_start(pps[:], pp_d[:, :])

            # ---------------- gating + projection + stats ----------------
            with (
                tc.tile_pool(name="gin", bufs=GIN_BUFS) as gin,
                tc.tile_pool(name="gw", bufs=GW_BUFS) as gw,
                tc.tile_pool(name="gate", bufs=GATE_BUFS) as gatep,
                tc.tile_pool(name="pg", bufs=PG_BUFS, space="PSUM") as pg,
                tc.tile_pool(name="po", bufs=PO_BUFS, space="PSUM") as po,
            ):
                for i in range(NT):
                    sl = slice(i * F, (i + 1) * F)
                    a = [gin.tile([128, F], f32, tag=f"a{j}", name=f"a{j}") for j in (0, 1)]
                    l = [gin.tile([128, F], f32, tag=f"l{j}", name=f"l{j}") for j in (0, 1)]
                    for j in (0, 1):
                        nc.sync.dma_start(a[j][:], at_d[j, :, sl])
                        nc.sync.dma_start(l[j][:], lt_d[j, :, sl])
                    ar = [gw.tile([128, F], f32r, tag=f"ar{j}", name=f"ar{j}") for j in (0, 1)]
                    lr = [gw.tile([128, F], f32r, tag=f"lr{j}", name=f"lr{j}") for j in (0, 1)]
                    for j in (0, 1):
                        if AR_ENGINE == "act":
                            nc.scalar.activation(ar[j][:], a[j][:], Act.Copy)
                        elif AR_ENGINE == "dve":
                            nc.vector.tensor_copy(ar[j][:], a[j][:])
                        else:
                            nc.gpsimd.tensor_copy(ar[j][:], a[j][:])
                        if LR_ENGINE == "dve":
                            nc.vector.tensor_copy(lr[j][:], l[j][:])
                        else:
                            nc.gpsimd.tensor_copy(lr[j][:], l[j][:])
                    gL, gA = [], []
                    for j in (0, 1):
                        p1 = pg.tile([128, F], f32, tag=f"g1{j}", name=f"pg1{j}")
                        nc.tensor.matmul(p1[:], w1s[:, 0, j, :], ar[0][:],
                                         start=True, stop=False)
                        nc.tensor.matmul(p1[:], w1s[:, 1, j, :], ar[1][:],
                                         start=False, stop=True)
                        g = gatep.tile([128, F], f32, tag=f"gL{j}", name=f"gL{j}")
                        nc.scalar.activation(g[:], p1[:], Act.Sigmoid,
                                             bias=pps[:, 0 + j:1 + j])
                        gL.append(g)
                    for j in (0, 1):
                        p2 = pg.tile([128, F], f32, tag=f"g2{j}", name=f"pg2{j}")
                        nc.tensor.matmul(p2[:], w2s[:, 0, j, :], lr[0][:],
                                         start=True, stop=False)
                        nc.tensor.matmul(p2[:], w2s[:, 1, j, :], lr[1][:],
                                         start=False, stop=True)
                        g = gatep.tile([128, F], f32, tag=f"gA{j}", name=f"gA{j}")
                        nc.scalar.activation(g[:], p2[:], Act.Sigmoid,
                                             bias=pps[:, 2 + j:3 + j])
                        gA.append(g)
                    fu = []
                    for j in (0, 1):
                        v1 = gw.tile([128, F], f32, tag=f"v1{j}", name=f"v1{j}")
                        v2 = gw.tile([128, F], f32, tag=f"v2{j}", name=f"v2{j}")
                        nc.vector.tensor_mul(v1[:], a[j][:], gA[j][:])
                        nc.gpsimd.tensor_mul(v2[:], l[j][:], gL[j][:])
                        nc.vector.tensor_add(v1[:], v1[:], v2[:])
                        fu.append(v1)
                    for j in (0, 1):
                        p3 = po.tile([128, F], f32, tag=f"o{j}", name=f"po{j}")
                        nc.tensor.matmul(p3[:], w3s[:, 0, j, :], fu[0][:],
                                         start=True, stop=False)
                        nc.tensor.matmul(p3[:], w3s[:, 1, j, :], fu[1][:],
                                         start=False, stop=True)
                        nc.scalar.activation(outb[:, j, sl], p3[:],
                                             Act.Identity,
                                             bias=pps[:, 4 + j:5 + j])
                        if BN_FROM_PSUM:
                            nc.vector.bn_stats(stb[:, j, i, :], p3[:])
                        else:
                            nc.vector.bn_stats(stb[:, j, i, :], outb[:, j, sl])

            # ---------------- stats finalize + all-reduce ----------------
            if not do_fin:
                continue
            with (
                tc.tile_pool(name="fin", bufs=1) as fin,
                tc.tile_pool(name="dramp", bufs=1, space="DRAM") as dramp,
            ):
                mv = fin.tile([128, 2, 2], f32)
                ccs = fin.tile([128, 4], f32)
                for j in (0, 1):
                    nc.vector.bn_aggr(mv[:, j, :], stb[:, j, :, :])
                    if BN_FROM_PSUM:
                        nc.vector.tensor_add(mv[:, j, 0:1], mv[:, j, 0:1],
                                             pps[:, 4 + j:5 + j])
                    nc.vector.tensor_copy(ccs[:, 2 * j:2 * j + 1], mv[:, j, 0:1])
                    nc.vector.tensor_mul(ccs[:, 2 * j + 1:2 * j + 2],
                                         mv[:, j, 0:1], mv[:, j, 0:1])
                    nc.vector.tensor_add(ccs[:, 2 * j + 1:2 * j + 2],
                                         ccs[:, 2 * j + 1:2 * j + 2],
                                         mv[:, j, 1:2])
                if single_core:
                    cg = fin.tile([128, 4], f32)
                    nc.vector.tensor_scalar(cg[:], ccs[:], float(NCORES),
                                            None, Alu.mult)
                else:
                    cc_in = dramp.tile([128, 4], f32)
                    cc_out = dramp.tile([128, 4], f32)
                    nc.gpsimd.dma_start(cc_in[:], ccs[:])
                    nc.gpsimd.collective_compute(
                        "AllReduce", Alu.add,
                        replica_groups=[list(range(NCORES))],
                        ins=[cc_in.opt()], outs=[cc_out.opt()],
                    )
                    cg = fin.tile([128, 4], f32)
                    nc.gpsimd.dma_start(cg[:], cc_out[:])

                mean = fin.tile([128, 2], f32)
                varp = fin.tile([128, 2], f32)
                sc = fin.tile([128, 2], f32)
                sh = fin.tile([128, 2], f32)
                t1 = fin.tile([128, 2], f32)
                t2 = fin.tile([128, 2], f32)
                r0 = fin.tile([128, 2], f32)
                for j in (0, 1):
                    jm = slice(j, j + 1)
                    nc.vector.tensor_scalar(mean[:, jm], cg[:, 2 * j:2 * j + 1],
                                            1.0 / NCORES, None, Alu.mult)
                    # varp = E[x^2] - mean^2 + eps
                    nc.vector.tensor_scalar(varp[:, jm],
                                            cg[:, 2 * j + 1:2 * j + 2],
                                            1.0 / NCORES, None, Alu.mult)
                    nc.vector.tensor_mul(t1[:, jm], mean[:, jm], mean[:, jm])
                    nc.vector.tensor_sub(varp[:, jm], varp[:, jm], t1[:, jm])
                    nc.vector.tensor_scalar(varp[:, jm], varp[:, jm], EPS,
                                            None, Alu.add)
                # r0 = 1/sqrt(varp), via ACT sqrt + reciprocal + 2 Newton steps
                nc.scalar.activation(r0[:], varp[:], Act.Sqrt)
                nc.vector.reciprocal(r0[:], r0[:])
                for _ in range(2):
                    nc.vector.tensor_mul(t1[:], r0[:], r0[:])
                    nc.vector.tensor_mul(t2[:], t1[:], varp[:])
                    nc.vector.tensor_scalar(t2[:], t2[:], -0.5, 1.5,
                                            Alu.mult, Alu.add)
                    nc.vector.tensor_mul(r0[:], r0[:], t2[:])
                for j in (0, 1):
                    jm = slice(j, j + 1)
                    nc.vector.tensor_mul(sc[:, jm], pps[:, 6 + j:7 + j],
                                         r0[:, jm])
                    nc.vector.tensor_mul(t1[:, jm], mean[:, jm], sc[:, jm])
                    nc.vector.tensor_sub(sh[:, jm], pps[:, 8 + j:9 + j],
                                         t1[:, jm])

                # per-t pre-scaled BN affine: ay_t = alpha_t*(sc*out+sh)
                asc = fin.tile([128, T, 2], f32)
                ash = fin.tile([128, T, 2], f32)
                for t in range(T):
                    for j in (0, 1):
                        nc.vector.tensor_scalar(asc[:, t, j:j + 1], sc[:, j:j + 1],
                                                alphas[t], None, Alu.mult)
                        nc.vector.tensor_scalar(ash[:, t, j:j + 1], sh[:, j:j + 1],
                                                alphas[t], None, Alu.mult)

                # ---------------- LIF scan + spike output ----------------
                if not do_lif:
                    continue
                with tc.tile_pool(name="lif", bufs=LIF_BUFS) as lifp:
                    for q in range(nq):
                        for j in (0, 1):
                            veng = nc.gpsimd if q in LIF_POOL_Q else nc.vector
                            seng = nc.vector if S_ON_DVE else nc.gpsimd
                            wprev = None
                            for t in range(T):
                                ysl = outb[:, j, t * RT + q * cc:
                                           t * RT + (q + 1) * cc]
                                ay = lifp.tile([128, cc], f32, tag=f"ay{j}", name=f"ay{j}", bufs=AY_BUFS)
                                nc.scalar.activation(
                                    ay[:], ysl, Act.Identity,
                                    bias=ash[:, t, j:j + 1],
                                    scale=asc[:, t, j:j + 1])
                                if t == 0:
                                    u = ay
                                else:
                                    # U = ay - Wn  (Wn = -W)
                                    u = lifp.tile([128, cc], f32, tag=f"U{j}", name=f"U{j}")
                                    veng.tensor_sub(u[:], ay[:], wprev[:])
                                s = lifp.tile([128, cc], f32, tag=f"s{j}", name=f"s{j}")
                                seng.tensor_scalar(
                                    s[:], u[:], ths[t], None, Alu.is_ge)
                                nc.sync.dma_start(
                                    sp_d[j, :, t * RT + q * cc:
                                         t * RT + (q + 1) * cc], s[:])
                                if t < T - 1:
                                    wn = lifp.tile([128, cc], f32, tag=f"W{j}", name=f"Wn{j}")
                                    veng.scalar_tensor_tensor(
                                        wn[:], s[:], 1.0, u[:],
                                        Alu.subtract, Alu.mult)
                                    wprev = wn

    nc.compile()
    return nc


def _get_program(tau_inv: float, reps: int = 1, single_core: bool = False):
    key = (round(float(tau_inv), 12), reps, single_core)
    if key not in _program_cache:
        _program_cache[key] = _build_program(float(tau_inv), reps, single_core)
    return _program_cache[key]


def _shard_transpose(x):
    # [T,B,N,C] -> [cores, 2, 128, R] with rows ordered (t, b_loc, n)
    v = x.reshape(T, NCORES, BL, N, C)
    v = np.transpose(v, (1, 4, 0, 2, 3))
    return np.ascontiguousarray(v).reshape(NCORES, 2, 128, R)


def _prep_w(w):
    # lhsT chunks [p, k, j, q]: W.T viewed as [k,128p][j,128q]
    wt = np.ascontiguousarray(w.T).reshape(2, 128, 2, 128)
    return np.ascontiguousarray(wt.transpose(1, 0, 2, 3))


def _two(vec):
    return np.ascontiguousarray(vec.reshape(2, 128).T)


def _make_in_maps(inputs):
    x_attn = np.asarray(inputs["x_attn"], dtype=np.float32)
    x_lsm = np.asarray(inputs["x_lsm"], dtype=np.float32)
    at = _shard_transpose(x_attn)
    lt = _shard_transpose(x_lsm)
    w1 = _prep_w(np.asarray(inputs["W_att"], dtype=np.float32))
    w2 = _prep_w(np.asarray(inputs["W_lsm"], dtype=np.float32))
    w3 = _prep_w(np.asarray(inputs["W_proj"], dtype=np.float32))
    pp = np.concatenate(
        [_two(np.asarray(inputs["b_att"], dtype=np.float32)),
         _two(np.asarray(inputs["b_lsm"], dtype=np.float32)),
         _two(np.asarray(inputs["b_proj"], dtype=np.float32)),
         _two(np.asarray(inputs["gamma"], dtype=np.float32)),
         _two(np.asarray(inputs["beta"], dtype=np.float32))],
        axis=1)
    return [
        {"at": at[s], "lt": lt[s], "w1": w1, "w2": w2, "w3": w3, "pp": pp}
        for s in range(NCORES)
    ]


def kernel(**inputs):
    from concourse.bass_utils import run_bass_kernel_spmd

    lif_w = float(np.asarray(inputs["lif_w"], dtype=np.float32))
    tau_inv = float(np.float32(1.0 / (1.0 + math.exp(-lif_w))))
    nc = _get_program(tau_inv)
    in_maps = _make_in_maps(inputs)
    res = run_bass_kernel_spmd(nc, in_maps, core_ids=list(range(NCORES)))
    kernel.last_results = res

    S = np.stack([r["sp"] for r in res.results]).reshape(
        NCORES, 2, 128, T, BL, N)
    out = np.transpose(S, (3, 0, 4, 5, 1, 2))
    return np.ascontiguousarray(out).reshape(T, B, N, C)
